# revision 14
# baseline (speedup 1.0000x reference)
"""LSTM (H=32, input-size 1) over B=32, T=16384 on 8 TRN2 NeuronCores.

Strategy: pure data parallel over batch (4 rows per core). Within a core,
the sequential recurrence is evaluated chunk-by-chunk with Jacobi (Picard)
iterations — DEER-style parallel-in-time evaluation:

  * chunk of K timesteps, J fixed-point sweeps per chunk
  * gate pre-activations accumulate in PSUM:  raw += W_bd @ dH  (the
    stationary operand is a block-diagonal [128,128] replication of the
    32x32 per-gate recurrent weight over the 4 local batch rows, so ONE
    matmul per gate covers all batches and lands directly in the
    (batch,hidden)-partition layout used by the elementwise engines)
  * the chunk is processed in 256-column segments so PE/ACT/DVE pipeline
    against each other instead of serializing per sweep
  * g-gate weights are pre-scaled by 2 host-side, so ONE sigmoid covers
    all four gates (tanh(g) = 2*sigmoid(2g)-1); the correction folds into
    the fused (sig_g - 0.5)*sig_i DVE op, which computes m/2 — the c
    recurrence then runs at half scale and tanh(c) = tanh(2*(c/2)) uses
    the activation's free input scale
  * the c-recurrence c_t = f_t*c_{t-1} + m_t over a whole segment is ONE
    DVE tensor_tensor_scan instruction, chained across segments
  * convergence is geometric (~10x per sweep) and chunk-size independent
    (measured), so J=8 reaches ~2e-5.

Everything (weight block-diagonalization, gate reorder to [i,f,o,g],
bias folding into the x-injection matmul) is precomputed host-side.
"""

import os
import numpy as np

import concourse.bass as bass
import concourse.bacc as bacc
import concourse.tile as tile
import concourse.mybir as mybir
from concourse.bass_utils import run_bass_kernel_spmd

H = 32
B = 32
T = 16384
NCORES = 8
BL = B // NCORES          # batch rows per core = 4
P = BL * H                # 128 partitions = (batch, hidden)

K = int(os.environ.get("LSTM_K", "512"))    # chunk length (timesteps)
J = int(os.environ.get("LSTM_J", "8"))      # Jacobi sweeps per chunk
SW = int(os.environ.get("LSTM_SW", "256"))  # segment width (>=256 for f32r)
MM = os.environ.get("LSTM_MM", "f32r")      # matmul operand dtype: f32r | f32

F32 = mybir.dt.float32
F32R = mybir.dt.float32r
MMDT = F32R if MM == "f32r" else F32
AF = mybir.ActivationFunctionType
OP = mybir.AluOpType


def build_nc(k=K, j_iters=J, t_total=T, sw=SW):
    nc = bacc.Bacc("TRN2", target_bir_lowering=False, debug=False)

    n_chunks = t_total // k
    assert t_total % k == 0 and k % sw == 0
    nsg = k // sw                         # segments per chunk
    GW = 4 * sw                           # columns per segment group in raw

    x_d = nc.declare_dram_parameter("x", [BL, t_total], MMDT, isOutput=False)
    wbd_d = nc.declare_dram_parameter("wbd", [P, 4 * P], MMDT, isOutput=False)
    rj_d = nc.declare_dram_parameter("rj", [2 * BL, 4 * P], MMDT, isOutput=False)
    wo_d = nc.declare_dram_parameter("wo", [P, BL], MMDT, isOutput=False)
    bo_d = nc.declare_dram_parameter("bo", [BL, 1], F32, isOutput=False)
    y_d = nc.declare_dram_parameter("y", [BL, t_total], F32, isOutput=True)

    with tile.TileContext(nc) as tc:
        with (
            tc.tile_pool(name="const", bufs=1) as cpool,
            tc.tile_pool(name="state", bufs=1) as spool,
            tc.tile_pool(name="work", bufs=2) as wpool,
            tc.tile_pool(name="praw", bufs=1, space="PSUM") as praw,
            tc.tile_pool(name="py", bufs=1, space="PSUM") as pypool,
        ):
            # ---- constants ----
            wbd = cpool.tile([P, 4 * P], MMDT)
            rj = cpool.tile([2 * BL, 4 * P], MMDT)
            wo = cpool.tile([P, BL], MMDT)
            bo = cpool.tile([BL, 1], F32)
            zrow = cpool.tile([1, P], MMDT)
            nc.vector.memset(zrow[:].bitcast(F32), 0.0)
            nc.sync.dma_start(wbd[:], wbd_d[:])
            nc.sync.dma_start(rj[:], rj_d[:])
            nc.sync.dma_start(wo[:], wo_d[:])
            nc.sync.dma_start(bo[:], bo_d[:])

            # ---- persistent state ----
            hbufs = [spool.tile([P, k + 1], MMDT, tag=t, name=t) for t in ("hA", "hB")]
            dlt = spool.tile([P, k], MMDT)       # h^{j} - h^{j-1}
            ccar = spool.tile([P, 1], F32)       # c/2 carry-in for the chunk

            nc.vector.memset(hbufs[0][:].bitcast(F32), 0.0)
            nc.vector.memset(hbufs[1][:].bitcast(F32), 0.0)
            nc.vector.memset(ccar[:], 0.0)

            # PSUM raw gates, per-segment interleaved: [i|f|o|g] x nsg
            raw = praw.tile([P, 4 * k], F32)

            def blk(s, g):
                return slice(s * GW + g * sw, s * GW + (g + 1) * sw)

            for n in range(n_chunks):
                # ---- per-chunk input: X rows (x_b at 2b, ones at 2b+1) ----
                xt = wpool.tile([2 * BL, k], MMDT, tag="xt")
                nc.vector.memset(xt[:].bitcast(F32), 1.0)
                for b in range(BL):
                    nc.sync.dma_start(
                        xt[2 * b : 2 * b + 1, :], x_d[b : b + 1, n * k : (n + 1) * k]
                    )

                if n > 0:
                    # zero the guess (cols 1..K of hA); col 0 holds the carry
                    nc.gpsimd.memset(hbufs[0][:, 1 : k + 1].bitcast(F32), 0.0)

                # ---- zero raw banks (start=True arms a full 2KB bank, so
                # the only start-bit writes are these full-bank zero matmuls;
                # everything else accumulates) ----
                for bk in range(4 * k // 512):
                    nc.tensor.matmul(
                        raw[:, bk * 512 : (bk + 1) * 512],
                        zrow[:], wbd[0:1, 0:512],
                        start=True, stop=False, skip_group_check=True,
                    )

                # ---- x/bias injection: raw += Rg^T @ X ----
                for s in range(nsg):
                    for g in range(4):
                        nc.tensor.matmul(
                            raw[:, blk(s, g)],
                            rj[:, g * P : (g + 1) * P],
                            xt[:, s * sw : (s + 1) * sw],
                            start=False, stop=False, skip_group_check=True,
                        )

                # ---- Jacobi sweeps ----
                for j in range(1, j_iters + 1):
                    gbuf = hbufs[(j - 1) % 2]
                    nbuf = hbufs[j % 2]
                    rhs = gbuf[:, 0:k] if j == 1 else dlt[:]

                    sig = wpool.tile([P, 4 * k], F32, tag="sig")
                    c = wpool.tile([P, k], F32, tag="c")
                    m = wpool.tile([P, k], F32, tag="m")
                    tau = wpool.tile([P, k], F32, tag="tau")

                    for s in range(nsg):
                        for g in range(4):
                            nc.tensor.matmul(
                                raw[:, blk(s, g)],
                                wbd[:, g * P : (g + 1) * P],
                                rhs[:, s * sw : (s + 1) * sw],
                                start=False, stop=(j == j_iters),
                                skip_group_check=True,
                            )

                        # one sigmoid over [i|f|o|g*2] of this segment
                        nc.scalar.activation(
                            sig[:, s * GW : (s + 1) * GW],
                            raw[:, s * GW : (s + 1) * GW], AF.Sigmoid)

                        i_s = sig[:, blk(s, 0)]
                        f_s = sig[:, blk(s, 1)]
                        o_s = sig[:, blk(s, 2)]
                        sg_s = sig[:, blk(s, 3)]
                        m_s = m[:, s * sw : (s + 1) * sw]
                        c_s = c[:, s * sw : (s + 1) * sw]

                        # m/2 = (sig(2g) - 0.5) * sig(i)   [tanh folded]
                        nc.vector.scalar_tensor_tensor(
                            m_s, sg_s, 0.5, i_s, OP.subtract, OP.mult)

                        init = ccar[:] if s == 0 else c[:, s * sw - 1 : s * sw]
                        nc.vector.tensor_tensor_scan(
                            c_s, f_s, m_s, init, OP.mult, OP.add)

                        # tanh(c) = tanh(2 * (c/2)) via free input scale
                        nc.scalar.activation(
                            tau[:, s * sw : (s + 1) * sw], c_s, AF.Tanh, scale=2.0)

                        nc.vector.tensor_mul(
                            nbuf[:, 1 + s * sw : 1 + (s + 1) * sw],
                            o_s, tau[:, s * sw : (s + 1) * sw])

                        if j < j_iters:
                            nc.vector.tensor_sub(
                                dlt[:, s * sw : (s + 1) * sw],
                                nbuf[:, s * sw : (s + 1) * sw],
                                gbuf[:, s * sw : (s + 1) * sw])

                fin = hbufs[j_iters % 2]

                # ---- output projection y = W_out @ h + b_out ----
                yp = pypool.tile([BL, k], F32)
                for s in range(0, k, 512):
                    w = min(512, k - s)
                    nc.tensor.matmul(
                        yp[:, s : s + w], wo[:], fin[:, 1 + s : 1 + s + w],
                        start=True, stop=True)
                ysb = wpool.tile([BL, k], F32, tag="ysb")
                nc.scalar.activation(ysb[:], yp[:, 0:k], AF.Identity, bias=bo[:])
                nc.sync.dma_start(y_d[:, n * k : (n + 1) * k], ysb[:])

                # ---- carries for next chunk ----
                if n < n_chunks - 1:
                    nc.vector.tensor_copy(hbufs[0][:, 0:1], fin[:, k : k + 1])
                    nc.vector.tensor_copy(hbufs[1][:, 0:1], fin[:, k : k + 1])
                    nc.vector.tensor_copy(ccar[:], c[:, k - 1 : k])

    nc.compile()
    return nc


def _host_precompute(W_ih, W_hh, b_ih, b_hh, W_out, b_out):
    """Block-diagonal stationary operands; gate order -> [i,f,o,g];
    g-gate rows pre-scaled by 2 (tanh-via-sigmoid folding)."""
    perm = np.concatenate([np.arange(0, 32), np.arange(32, 64),
                           np.arange(96, 128), np.arange(64, 96)])
    scale = np.ones((128, 1), np.float32)
    scale[96:] = 2.0                      # g block doubled
    Wh = W_hh[perm] * scale               # (128, 32)
    Wi = (W_ih[perm, 0:1] * scale)[:, 0]  # (128,)
    bs = (b_ih + b_hh)[perm] * scale[:, 0]

    wbd = np.zeros((P, 4 * P), np.float32)
    rj = np.zeros((2 * BL, 4 * P), np.float32)
    for g in range(4):
        Wg = Wh[g * 32 : (g + 1) * 32]    # (32, 32): [out_h, in_h]
        for b in range(BL):
            sl = slice(g * P + b * 32, g * P + b * 32 + 32)
            wbd[b * 32 : (b + 1) * 32, sl] = Wg.T
            rj[2 * b, sl] = Wi[g * 32 : (g + 1) * 32]
            rj[2 * b + 1, sl] = bs[g * 32 : (g + 1) * 32]

    wo = np.zeros((P, BL), np.float32)
    for b in range(BL):
        wo[b * 32 : (b + 1) * 32, b] = W_out[0]
    bo = np.full((BL, 1), np.float32(b_out[0]), np.float32)
    return wbd, rj, wo, bo


_NC_CACHE = {}


def _get_nc():
    key = (K, J, SW)
    if key not in _NC_CACHE:
        _NC_CACHE[key] = build_nc(K, J, T, SW)
    return _NC_CACHE[key]


def kernel(x, W_ih, W_hh, b_ih, b_hh, W_out, b_out):
    x = np.asarray(x, np.float32)
    wbd, rj, wo, bo = _host_precompute(
        np.asarray(W_ih, np.float32), np.asarray(W_hh, np.float32),
        np.asarray(b_ih, np.float32), np.asarray(b_hh, np.float32),
        np.asarray(W_out, np.float32), np.asarray(b_out, np.float32))

    xs = x[:, :, 0]                      # (B, T)
    in_maps = []
    for cidx in range(NCORES):
        in_maps.append({
            "x": np.ascontiguousarray(xs[cidx * BL : (cidx + 1) * BL]),
            "wbd": wbd, "rj": rj, "wo": wo, "bo": bo,
        })

    nc = _get_nc()
    res = run_bass_kernel_spmd(nc, in_maps, core_ids=list(range(NCORES)))
    ys = [res.results[cidx]["y"] for cidx in range(NCORES)]
    y = np.concatenate(ys, axis=0)       # (B, T)
    return y[:, :, None].astype(np.float32)
